# revision 5
# baseline (speedup 1.0000x reference)
"""Binarized 3x3 conv (XNOR-style): sign(conv2d(sign(x), sign(w)) + b).

Full-input contract: kernel(x=[32,256,56,56]f32, weight=[256,256,3,3]f32,
bias=[256]f32) -> [32,256,56,56]f32.

Strategy: data-parallel over batch across 8 NeuronCores (4 images/core).
Per core:
  - sign(x) encoded as +/-0.5 (exact: is_ge -> {0,1}, subtract 0.5) into a
    zero-padded 58x58 per-image layout, fp8e4 (or bf16).
  - sign(w) prepped on host as +/-1 in [c_partition, tap, (pair,) k] layout.
  - conv = 9 tap-shifted matmuls (fp8 DoubleRow, contract=256) accumulating
    into PSUM. All products are +/-0.5 with f32 accumulation, so
    psum == conv/2 exactly (conv is an even integer in [-2304, 2304]).
  - output sign = clamp(conv/2, -1, 1), exact for even integers including 0.
    One DVE tensor_scalar(min 1.0, max -1.0) per tile.
Bias is asserted zero (setup_inputs uses zeros); a nonzero bias falls back to
an exact 3-op sign path.
"""

import numpy as np

import concourse.bacc as bacc
import concourse.mybir as mybir
import concourse.tile as tile
from concourse.bass_utils import run_bass_kernel_spmd

N_CORES = 8
N_PER = 4          # images per core
C = 256            # input channels
K = 256            # output channels
H = W = 56
HP = WP = 58       # padded
XSP = HP * WP      # 3364 padded image pixels
HALF = 3376        # per-(image, pair-half) stride, padded to %16==0
RB = 8             # output rows per matmul tile
F = RB * WP        # 464 matmul free size (8 rows x 58, last 2 cols of each row garbage)
NBLK = H // RB     # 7 row blocks per image

USE_FP8 = True

_cache = {}


def _build(mode, with_bias):
    dt = mybir.dt
    xdt = dt.float8e4 if mode == "fp8" else dt.bfloat16
    nc = bacc.Bacc()
    x_d = nc.declare_dram_parameter("xs", [N_PER, C, H, W], dt.float32, isOutput=False)
    wfree = 9 * 2 * 256
    w_d = nc.declare_dram_parameter("wsgn", [128, wfree], xdt, isOutput=False)
    if with_bias:
        b_d = nc.declare_dram_parameter("bhalf", [128, 2], dt.float32, isOutput=False)
    o_d = nc.declare_dram_parameter("out", [N_PER, K, H, W], dt.float32, isOutput=True)

    with tile.TileContext(nc) as tc:
        with (
            tc.tile_pool(name="wpool", bufs=1) as wpool,
            tc.tile_pool(name="xsgn", bufs=N_PER) as xsgn_pool,
            tc.tile_pool(name="xf32", bufs=3) as xf_pool,
            tc.tile_pool(name="osb", bufs=6) as o_pool,
            tc.tile_pool(name="psum", bufs=8, space="PSUM") as p_pool,
        ):
            w_sb = wpool.tile([128, wfree], xdt)
            nc.sync.dma_start(w_sb[:], w_d[:])
            if with_bias:
                b_sb = wpool.tile([128, 2], dt.float32)
                nc.sync.dma_start(b_sb[:], b_d[:])

            # x sign tiles: one per image, both channel halves: [128, 2*HALF]
            # free index = ci*HALF + (y*58 + x) over the padded 58x58 grid.
            # Only the borders (and the 12-elem tail pad) need zeroing; the
            # 56x56 interior is overwritten by the sign writes.
            xs_tiles = []
            xv = x_d[:].rearrange("n c h w -> n c (h w)")
            RCH = 28  # rows per input DMA/sign chunk
            for n in range(N_PER):
                xs = xsgn_pool.tile([128, 2 * HALF], xdt, tag="xsgn")
                xs_tiles.append(xs)
                for ci in range(2):
                    grid = xs[:, ci * HALF: ci * HALF + XSP].rearrange(
                        "p (h w) -> p h w", h=HP
                    )
                    nc.gpsimd.memset(grid[:, 0, :], 0.0)           # top row
                    nc.gpsimd.memset(grid[:, HP - 1, :], 0.0)      # bottom row
                    # cols 0+57 of all interior rows: adjacent pairs (r,57),(r+1,0)
                    vert = xs[:, ci * HALF + 57: ci * HALF + 57 + 57 * WP].rearrange(
                        "p (h w) -> p h w", w=WP
                    )[:, :, 0:2]
                    nc.gpsimd.memset(vert, 0.0)
                    nc.gpsimd.memset(xs[:, ci * HALF + XSP: (ci + 1) * HALF], 0.0)
                for ch in range(H // RCH):
                    for ci in range(2):
                        xf = xf_pool.tile([128, RCH * W], dt.float32, tag="xf32")
                        nc.sync.dma_start(
                            xf[:],
                            xv[n, ci * 128:(ci + 1) * 128,
                               ch * RCH * W: (ch + 1) * RCH * W],
                        )
                        dst = (
                            xs[:, ci * HALF: ci * HALF + XSP]
                            .rearrange("p (h w) -> p h w", h=HP)
                            [:, 1 + ch * RCH: 1 + (ch + 1) * RCH, 1:57]
                        )
                        src = xf[:].rearrange("p (h w) -> p h w", h=RCH)
                        # (x>=0 -> {0,1}) - 0.5 = +/-0.5, exact
                        nc.vector.tensor_scalar(
                            dst, src, 0.0, 0.5, mybir.AluOpType.is_ge,
                            mybir.AluOpType.subtract,
                        )

            wv = w_sb[:].rearrange("p (t i k) -> p t i k", t=9, i=2)
            for kg in range(2):
                for n in range(N_PER):
                    xs = xs_tiles[n]
                    psums = [p_pool.tile([128, F], dt.float32, tag="ps", name=f"ps{kg}_{n}_{i}") for i in range(NBLK)]
                    if mode == "fp8":
                        xp = xs[:].rearrange("p (i f) -> p i f", i=2)
                        for tap in range(9):
                            ty, tx = tap // 3, tap % 3
                            lhsT = wv[:, tap, :, kg * 128:(kg + 1) * 128]
                            for rb in range(NBLK):
                                base = (rb * RB + ty) * WP + tx
                                rhs = xp[:, :, base: base + F]
                                nc.tensor.matmul(
                                    psums[rb][:], lhsT, rhs,
                                    start=(tap == 0), stop=(tap == 8),
                                    perf_mode=mybir.MatmulPerfMode.DoubleRow,
                                )
                    else:
                        step = 0
                        for ci in range(2):
                            for tap in range(9):
                                ty, tx = tap // 3, tap % 3
                                lhsT = wv[:, tap, ci, kg * 128:(kg + 1) * 128]
                                for rb in range(NBLK):
                                    base = ci * HALF + (rb * RB + ty) * WP + tx
                                    rhs = xs[:, base: base + F]
                                    nc.tensor.matmul(
                                        psums[rb][:], lhsT, rhs,
                                        start=(step == 0), stop=(step == 17),
                                    )
                                step += 1
                    for rb in range(NBLK):
                        # compact the valid 8x56 (of the 8x58 psum span) so
                        # the output DMA is contiguous on both sides
                        osb = o_pool.tile([128, RB * W], dt.float32, tag="osb")
                        psv = psums[rb][:].rearrange(
                            "p (r c) -> p r c", r=RB)[:, :, 0:W]
                        ov = osb[:].rearrange("p (r c) -> p r c", r=RB)
                        if not with_bias:
                            # exact sign of even integers: clamp(v/2, -1, 1)
                            nc.vector.tensor_scalar(
                                ov, psv, 1.0, -1.0,
                                mybir.AluOpType.min, mybir.AluOpType.max,
                            )
                        else:
                            # exact sign(v + b): (v/2+b/2 > 0) - (v/2+b/2 < 0)
                            tpos = o_pool.tile([128, RB * W], dt.float32, tag="tpos")
                            tneg = o_pool.tile([128, RB * W], dt.float32, tag="tneg")
                            bcol = b_sb[:, kg: kg + 1]
                            nc.vector.tensor_scalar(
                                tpos[:].rearrange("p (r c) -> p r c", r=RB), psv,
                                bcol, 0.0,
                                mybir.AluOpType.add, mybir.AluOpType.is_gt,
                            )
                            nc.vector.tensor_scalar(
                                tneg[:].rearrange("p (r c) -> p r c", r=RB), psv,
                                bcol, 0.0,
                                mybir.AluOpType.add, mybir.AluOpType.is_lt,
                            )
                            nc.vector.tensor_tensor(
                                osb[:], tpos[:], tneg[:], mybir.AluOpType.subtract,
                            )
                        dst = o_d[n, kg * 128:(kg + 1) * 128, rb * RB: rb * RB + RB, :]
                        nc.sync.dma_start(dst, osb[:])

    nc.finalize()
    return nc


def _prep_weights(weight, mode):
    dt = mybir.dt
    xdt = dt.float8e4 if mode == "fp8" else dt.bfloat16
    sgn = np.sign(weight.astype(np.float32))
    w4 = sgn.reshape(K, 2, 128, 3, 3)          # [k, i, p, ty, tx]
    arr = w4.transpose(2, 3, 4, 1, 0)          # [p, ty, tx, i, k]
    arr = np.ascontiguousarray(arr).reshape(128, 9 * 2 * 256)
    return arr.astype(mybir.dt.np(xdt))


def kernel(x, weight, bias, _profile=False, _trace_kwargs=None):
    mode = "fp8" if USE_FP8 else "bf16"
    x = np.asarray(x, dtype=np.float32)
    weight = np.asarray(weight, dtype=np.float32)
    bias = np.asarray(bias, dtype=np.float32)
    with_bias = bool(np.any(bias != 0.0))

    key = (mode, with_bias)
    if key not in _cache:
        _cache[key] = _build(mode, with_bias)
    nc = _cache[key]

    wsgn = _prep_weights(weight, mode)
    in_maps = []
    for c in range(N_CORES):
        m = {
            "xs": np.ascontiguousarray(x[c * N_PER:(c + 1) * N_PER]),
            "wsgn": wsgn,
        }
        if with_bias:
            m["bhalf"] = np.ascontiguousarray(
                (bias.reshape(2, 128).T * 0.5).astype(np.float32)
            )
        in_maps.append(m)

    res = run_bass_kernel_spmd(
        nc, in_maps, core_ids=list(range(N_CORES)),
        trace=_profile, **(_trace_kwargs or {}),
    )
    out = np.concatenate([res.results[c]["out"] for c in range(N_CORES)], axis=0)
    if _profile:
        kernel.last_exec_ns = res.exec_time_ns
        kernel.last_results = res
    return out


# revision 9
# speedup vs baseline: 1.0776x; 1.0776x over previous
"""Binarized 3x3 conv (XNOR-style): sign(conv2d(sign(x), sign(w)) + b).

Full-input contract: kernel(x=[32,256,56,56]f32, weight=[256,256,3,3]f32,
bias=[256]f32) -> [32,256,56,56]f32.

Strategy: data-parallel over batch across 8 NeuronCores (4 images/core).
Per core:
  - sign(x) encoded as +/-0.5 (exact: is_ge -> {0,1}, subtract 0.5) into a
    zero-padded 58x58 per-image layout, fp8e4 (or bf16).
  - sign(w) prepped on host as +/-1 in [c_partition, tap, (pair,) k] layout.
  - conv = 9 tap-shifted matmuls (fp8 DoubleRow, contract=256) accumulating
    into PSUM. All products are +/-0.5 with f32 accumulation, so
    psum == conv/2 exactly (conv is an even integer in [-2304, 2304]).
  - output sign = clamp(conv/2, -1, 1), exact for even integers including 0.
    One DVE tensor_scalar(min 1.0, max -1.0) per tile.
Bias is asserted zero (setup_inputs uses zeros); a nonzero bias falls back to
an exact 3-op sign path.
"""

import numpy as np

import concourse.bacc as bacc
import concourse.mybir as mybir
import concourse.tile as tile
from concourse.bass_utils import run_bass_kernel_spmd

N_CORES = 8
N_PER = 4          # images per core
C = 256            # input channels
K = 256            # output channels
H = W = 56
HP = WP = 58       # padded
XSP = HP * WP      # 3364 padded image pixels
HALF = 3376        # per-(image, pair-half) stride, padded to %16==0
RB = 8             # output rows per matmul tile
F = RB * WP        # 464 matmul free size (8 rows x 58, last 2 cols of each row garbage)
NBLK = H // RB     # 7 row blocks per image

USE_FP8 = True

_cache = {}


def _build(mode, with_bias):
    dt = mybir.dt
    xdt = dt.float8e4 if mode == "fp8" else dt.bfloat16
    nc = bacc.Bacc()
    x_d = nc.declare_dram_parameter("xs", [N_PER, C, H, W], dt.float32, isOutput=False)
    wfree = 9 * 2 * 256
    w_d = nc.declare_dram_parameter("wsgn", [128, wfree], xdt, isOutput=False)
    if with_bias:
        b_d = nc.declare_dram_parameter("bhalf", [128, 2], dt.float32, isOutput=False)
    o_d = nc.declare_dram_parameter("out", [N_PER, K, H, W], dt.float32, isOutput=True)

    with tile.TileContext(nc) as tc:
        with (
            tc.tile_pool(name="wpool", bufs=1) as wpool,
            tc.tile_pool(name="xsgn", bufs=N_PER) as xsgn_pool,
            tc.tile_pool(name="xf32", bufs=3) as xf_pool,
            tc.tile_pool(name="osb", bufs=6) as o_pool,
            tc.tile_pool(name="psum", bufs=8, space="PSUM") as p_pool,
        ):
            w_sb = wpool.tile([128, wfree], xdt)
            nc.sync.dma_start(w_sb[:], w_d[:])
            if with_bias:
                b_sb = wpool.tile([128, 2], dt.float32)
                nc.sync.dma_start(b_sb[:], b_d[:])

            # Warm the PE HAM clock gate (~3.4us of activity -> 2.4 GHz)
            # while the first image is still streaming in. Results discarded.
            warm = p_pool.tile([128, F], dt.float32, tag="ps")
            for _ in range(14):
                nc.tensor.matmul(
                    warm[:], w_sb[:, 0:128], w_sb[:, 0:F],
                    start=True, stop=True,
                )

            # x sign tiles: one per image, both channel halves: [128, 2*HALF]
            # free index = ci*HALF + (y*58 + x) over the padded 58x58 grid.
            # Only the borders (and the 12-elem tail pad) need zeroing; the
            # 56x56 interior is overwritten by the sign writes.
            xs_tiles = []
            xv = x_d[:].rearrange("n c h w -> n c (h w)")
            RCH = 28  # rows per input DMA/sign chunk
            for n in range(N_PER):
                xs = xsgn_pool.tile([128, 2 * HALF], xdt, tag="xsgn")
                xs_tiles.append(xs)
                for ci in range(2):
                    grid = xs[:, ci * HALF: ci * HALF + XSP].rearrange(
                        "p (h w) -> p h w", h=HP
                    )
                    nc.gpsimd.memset(grid[:, 0, :], 0.0)           # top row
                    nc.gpsimd.memset(grid[:, HP - 1, :], 0.0)      # bottom row
                    # cols 0+57 of all interior rows: adjacent pairs (r,57),(r+1,0)
                    vert = xs[:, ci * HALF + 57: ci * HALF + 57 + 57 * WP].rearrange(
                        "p (h w) -> p h w", w=WP
                    )[:, :, 0:2]
                    nc.gpsimd.memset(vert, 0.0)
                    nc.gpsimd.memset(xs[:, ci * HALF + XSP: (ci + 1) * HALF], 0.0)
                for ch in range(H // RCH):
                    for ci in range(2):
                        xf = xf_pool.tile([128, RCH * W], dt.float32, tag="xf32")
                        nc.sync.dma_start(
                            xf[:],
                            xv[n, ci * 128:(ci + 1) * 128,
                               ch * RCH * W: (ch + 1) * RCH * W],
                        )
                        dst = (
                            xs[:, ci * HALF: ci * HALF + XSP]
                            .rearrange("p (h w) -> p h w", h=HP)
                            [:, 1 + ch * RCH: 1 + (ch + 1) * RCH, 1:57]
                        )
                        src = xf[:].rearrange("p (h w) -> p h w", h=RCH)
                        # (x>=0 -> {0,1}) - 0.5 = +/-0.5, exact
                        nc.vector.tensor_scalar(
                            dst, src, 0.0, 0.5, mybir.AluOpType.is_ge,
                            mybir.AluOpType.subtract,
                        )

            wv = w_sb[:].rearrange("p (t i k) -> p t i k", t=9, i=2)
            for kg in range(2):
                for n in range(N_PER):
                    xs = xs_tiles[n]
                    psums = [p_pool.tile([128, F], dt.float32, tag="ps", name=f"ps{kg}_{n}_{i}") for i in range(NBLK)]
                    # rb 0-2 first: those only need the first input row-chunk,
                    # so the PE can start before the whole image is signed
                    if mode == "fp8":
                        xp = xs[:].rearrange("p (i f) -> p i f", i=2)
                        for grp in (range(0, 3), range(3, NBLK)):
                            for tap in range(9):
                                ty, tx = tap // 3, tap % 3
                                lhsT = wv[:, tap, :, kg * 128:(kg + 1) * 128]
                                for rb in grp:
                                    base = (rb * RB + ty) * WP + tx
                                    rhs = xp[:, :, base: base + F]
                                    nc.tensor.matmul(
                                        psums[rb][:], lhsT, rhs,
                                        start=(tap == 0), stop=(tap == 8),
                                        perf_mode=mybir.MatmulPerfMode.DoubleRow,
                                    )
                    else:
                        for grp in (range(0, 3), range(3, NBLK)):
                            step = 0
                            for ci in range(2):
                                for tap in range(9):
                                    ty, tx = tap // 3, tap % 3
                                    lhsT = wv[:, tap, ci, kg * 128:(kg + 1) * 128]
                                    for rb in grp:
                                        base = ci * HALF + (rb * RB + ty) * WP + tx
                                        rhs = xs[:, base: base + F]
                                        nc.tensor.matmul(
                                            psums[rb][:], lhsT, rhs,
                                            start=(step == 0), stop=(step == 17),
                                        )
                                    step += 1
                    for rb in range(NBLK):
                        # compact the valid 8x56 (of the 8x58 psum span) so
                        # the output DMA is contiguous on both sides
                        osb = o_pool.tile([128, RB * W], dt.float32, tag="osb")
                        psv = psums[rb][:].rearrange(
                            "p (r c) -> p r c", r=RB)[:, :, 0:W]
                        ov = osb[:].rearrange("p (r c) -> p r c", r=RB)
                        if not with_bias:
                            # exact sign of even integers: clamp(v/2, -1, 1)
                            nc.vector.tensor_scalar(
                                ov, psv, 1.0, -1.0,
                                mybir.AluOpType.min, mybir.AluOpType.max,
                            )
                        else:
                            # exact sign(v + b): (v/2+b/2 > 0) - (v/2+b/2 < 0)
                            tpos = o_pool.tile([128, RB * W], dt.float32, tag="tpos")
                            tneg = o_pool.tile([128, RB * W], dt.float32, tag="tneg")
                            bcol = b_sb[:, kg: kg + 1]
                            nc.vector.tensor_scalar(
                                tpos[:].rearrange("p (r c) -> p r c", r=RB), psv,
                                bcol, 0.0,
                                mybir.AluOpType.add, mybir.AluOpType.is_gt,
                            )
                            nc.vector.tensor_scalar(
                                tneg[:].rearrange("p (r c) -> p r c", r=RB), psv,
                                bcol, 0.0,
                                mybir.AluOpType.add, mybir.AluOpType.is_lt,
                            )
                            nc.vector.tensor_tensor(
                                osb[:], tpos[:], tneg[:], mybir.AluOpType.subtract,
                            )
                        dst = o_d[n, kg * 128:(kg + 1) * 128, rb * RB: rb * RB + RB, :]
                        # stores go out via SWDGE (scalar engine) so they never
                        # queue ahead of the latency-critical input loads on
                        # the sync/HWDGE queues
                        nc.scalar.dma_start(dst, osb[:])

    nc.finalize()
    return nc


def _prep_weights(weight, mode):
    dt = mybir.dt
    xdt = dt.float8e4 if mode == "fp8" else dt.bfloat16
    sgn = np.sign(weight.astype(np.float32))
    w4 = sgn.reshape(K, 2, 128, 3, 3)          # [k, i, p, ty, tx]
    arr = w4.transpose(2, 3, 4, 1, 0)          # [p, ty, tx, i, k]
    arr = np.ascontiguousarray(arr).reshape(128, 9 * 2 * 256)
    return arr.astype(mybir.dt.np(xdt))


def kernel(x, weight, bias, _profile=False, _trace_kwargs=None):
    mode = "fp8" if USE_FP8 else "bf16"
    x = np.asarray(x, dtype=np.float32)
    weight = np.asarray(weight, dtype=np.float32)
    bias = np.asarray(bias, dtype=np.float32)
    with_bias = bool(np.any(bias != 0.0))

    key = (mode, with_bias)
    if key not in _cache:
        _cache[key] = _build(mode, with_bias)
    nc = _cache[key]

    wsgn = _prep_weights(weight, mode)
    in_maps = []
    for c in range(N_CORES):
        m = {
            "xs": np.ascontiguousarray(x[c * N_PER:(c + 1) * N_PER]),
            "wsgn": wsgn,
        }
        if with_bias:
            m["bhalf"] = np.ascontiguousarray(
                (bias.reshape(2, 128).T * 0.5).astype(np.float32)
            )
        in_maps.append(m)

    res = run_bass_kernel_spmd(
        nc, in_maps, core_ids=list(range(N_CORES)),
        trace=_profile, **(_trace_kwargs or {}),
    )
    out = np.concatenate([res.results[c]["out"] for c in range(N_CORES)], axis=0)
    if _profile:
        kernel.last_exec_ns = res.exec_time_ns
        kernel.last_results = res
    return out


# revision 10
# speedup vs baseline: 1.0961x; 1.0172x over previous
"""Binarized 3x3 conv (XNOR-style): sign(conv2d(sign(x), sign(w)) + b).

Full-input contract: kernel(x=[32,256,56,56]f32, weight=[256,256,3,3]f32,
bias=[256]f32) -> [32,256,56,56]f32.

Strategy: data-parallel over batch across 8 NeuronCores (4 images/core).
Per core:
  - sign(x) encoded as +/-0.5 (exact: is_ge -> {0,1}, subtract 0.5) into a
    zero-padded 58x58 per-image layout, fp8e4 (or bf16).
  - sign(w) prepped on host as +/-1 in [c_partition, tap, (pair,) k] layout.
  - conv = 9 tap-shifted matmuls (fp8 DoubleRow, contract=256) accumulating
    into PSUM. All products are +/-0.5 with f32 accumulation, so
    psum == conv/2 exactly (conv is an even integer in [-2304, 2304]).
  - output sign = clamp(conv/2, -1, 1), exact for even integers including 0.
    One DVE tensor_scalar(min 1.0, max -1.0) per tile.
Bias is asserted zero (setup_inputs uses zeros); a nonzero bias falls back to
an exact 3-op sign path.
"""

import numpy as np

import concourse.bacc as bacc
import concourse.mybir as mybir
import concourse.tile as tile
from concourse.bass_utils import run_bass_kernel_spmd

N_CORES = 8
N_PER = 4          # images per core
C = 256            # input channels
K = 256            # output channels
H = W = 56
HP = WP = 58       # padded
XSP = HP * WP      # 3364 padded image pixels
HALF = 3376        # per-(image, pair-half) stride, padded to %16==0
RB = 8             # output rows per matmul tile
F = RB * WP        # 464 matmul free size (8 rows x 58, last 2 cols of each row garbage)
NBLK = H // RB     # 7 row blocks per image

USE_FP8 = True

_cache = {}


def _build(mode, with_bias):
    dt = mybir.dt
    xdt = dt.float8e4 if mode == "fp8" else dt.bfloat16
    nc = bacc.Bacc()
    x_d = nc.declare_dram_parameter("xs", [N_PER, C, H, W], dt.float32, isOutput=False)
    wfree = 9 * 2 * 256
    w_d = nc.declare_dram_parameter("wsgn", [128, wfree], xdt, isOutput=False)
    if with_bias:
        b_d = nc.declare_dram_parameter("bhalf", [128, 2], dt.float32, isOutput=False)
    o_d = nc.declare_dram_parameter("out", [N_PER, K, H, W], dt.float32, isOutput=True)

    with tile.TileContext(nc) as tc:
        with (
            tc.tile_pool(name="wpool", bufs=1) as wpool,
            tc.tile_pool(name="xsgn", bufs=N_PER) as xsgn_pool,
            tc.tile_pool(name="xf32", bufs=3) as xf_pool,
            tc.tile_pool(name="osb", bufs=6) as o_pool,
            tc.tile_pool(name="psum", bufs=8, space="PSUM") as p_pool,
        ):
            w_sb = wpool.tile([128, wfree], xdt)
            nc.sync.dma_start(w_sb[:], w_d[:])
            if with_bias:
                b_sb = wpool.tile([128, 2], dt.float32)
                nc.sync.dma_start(b_sb[:], b_d[:])

            # Warm the PE HAM clock gate (~3.4us of activity -> 2.4 GHz)
            # while the first image is still streaming in. Results discarded.
            warm = p_pool.tile([128, F], dt.float32, tag="ps")
            for _ in range(14):
                nc.tensor.matmul(
                    warm[:], w_sb[:, 0:128], w_sb[:, 0:F],
                    start=True, stop=True,
                )

            # x sign tiles: one per image, both channel halves: [128, 2*HALF]
            # free index = ci*HALF + (y*58 + x) over the padded 58x58 grid.
            # Only the borders (and the 12-elem tail pad) need zeroing; the
            # 56x56 interior is overwritten by the sign writes.
            xs_tiles = []
            xv = x_d[:].rearrange("n c h w -> n c (h w)")
            RCH = 28  # rows per input DMA/sign chunk
            for n in range(N_PER):
                xs = xsgn_pool.tile([128, 2 * HALF], xdt, tag="xsgn")
                xs_tiles.append(xs)
                for ci in range(2):
                    grid = xs[:, ci * HALF: ci * HALF + XSP].rearrange(
                        "p (h w) -> p h w", h=HP
                    )
                    nc.gpsimd.memset(grid[:, 0, :], 0.0)           # top row
                    nc.gpsimd.memset(grid[:, HP - 1, :], 0.0)      # bottom row
                    # cols 0+57 of all interior rows: adjacent pairs (r,57),(r+1,0)
                    vert = xs[:, ci * HALF + 57: ci * HALF + 57 + 57 * WP].rearrange(
                        "p (h w) -> p h w", w=WP
                    )[:, :, 0:2]
                    nc.gpsimd.memset(vert, 0.0)
                    nc.gpsimd.memset(xs[:, ci * HALF + XSP: (ci + 1) * HALF], 0.0)
                for ch in range(H // RCH):
                    for ci in range(2):
                        xf = xf_pool.tile([128, RCH * W], dt.float32, tag="xf32")
                        nc.sync.dma_start(
                            xf[:],
                            xv[n, ci * 128:(ci + 1) * 128,
                               ch * RCH * W: (ch + 1) * RCH * W],
                        )
                        dst = (
                            xs[:, ci * HALF: ci * HALF + XSP]
                            .rearrange("p (h w) -> p h w", h=HP)
                            [:, 1 + ch * RCH: 1 + (ch + 1) * RCH, 1:57]
                        )
                        src = xf[:].rearrange("p (h w) -> p h w", h=RCH)
                        # (x>=0 -> {0,1}) - 0.5 = +/-0.5, exact
                        nc.vector.tensor_scalar(
                            dst, src, 0.0, 0.5, mybir.AluOpType.is_ge,
                            mybir.AluOpType.subtract,
                        )

            wv = w_sb[:].rearrange("p (t i k) -> p t i k", t=9, i=2)
            for n in range(N_PER):
                for kg in range(2):
                    xs = xs_tiles[n]
                    psums = [p_pool.tile([128, F], dt.float32, tag="ps", name=f"ps{kg}_{n}_{i}") for i in range(NBLK)]
                    # rb 0-2 first: those only need the first input row-chunk,
                    # so the PE can start before the whole image is signed
                    if mode == "fp8":
                        xp = xs[:].rearrange("p (i f) -> p i f", i=2)
                        for grp in (range(0, 3), range(3, NBLK)):
                            for tap in range(9):
                                ty, tx = tap // 3, tap % 3
                                lhsT = wv[:, tap, :, kg * 128:(kg + 1) * 128]
                                for rb in grp:
                                    base = (rb * RB + ty) * WP + tx
                                    rhs = xp[:, :, base: base + F]
                                    nc.tensor.matmul(
                                        psums[rb][:], lhsT, rhs,
                                        start=(tap == 0), stop=(tap == 8),
                                        perf_mode=mybir.MatmulPerfMode.DoubleRow,
                                    )
                    else:
                        for grp in (range(0, 3), range(3, NBLK)):
                            step = 0
                            for ci in range(2):
                                for tap in range(9):
                                    ty, tx = tap // 3, tap % 3
                                    lhsT = wv[:, tap, ci, kg * 128:(kg + 1) * 128]
                                    for rb in grp:
                                        base = ci * HALF + (rb * RB + ty) * WP + tx
                                        rhs = xs[:, base: base + F]
                                        nc.tensor.matmul(
                                            psums[rb][:], lhsT, rhs,
                                            start=(step == 0), stop=(step == 17),
                                        )
                                    step += 1
                    for rb in range(NBLK):
                        # compact the valid 8x56 (of the 8x58 psum span) so
                        # the output DMA is contiguous on both sides
                        osb = o_pool.tile([128, RB * W], dt.float32, tag="osb")
                        psv = psums[rb][:].rearrange(
                            "p (r c) -> p r c", r=RB)[:, :, 0:W]
                        ov = osb[:].rearrange("p (r c) -> p r c", r=RB)
                        if not with_bias:
                            # exact sign of even integers: clamp(v/2, -1, 1)
                            nc.vector.tensor_scalar(
                                ov, psv, 1.0, -1.0,
                                mybir.AluOpType.min, mybir.AluOpType.max,
                            )
                        else:
                            # exact sign(v + b): (v/2+b/2 > 0) - (v/2+b/2 < 0)
                            tpos = o_pool.tile([128, RB * W], dt.float32, tag="tpos")
                            tneg = o_pool.tile([128, RB * W], dt.float32, tag="tneg")
                            bcol = b_sb[:, kg: kg + 1]
                            nc.vector.tensor_scalar(
                                tpos[:].rearrange("p (r c) -> p r c", r=RB), psv,
                                bcol, 0.0,
                                mybir.AluOpType.add, mybir.AluOpType.is_gt,
                            )
                            nc.vector.tensor_scalar(
                                tneg[:].rearrange("p (r c) -> p r c", r=RB), psv,
                                bcol, 0.0,
                                mybir.AluOpType.add, mybir.AluOpType.is_lt,
                            )
                            nc.vector.tensor_tensor(
                                osb[:], tpos[:], tneg[:], mybir.AluOpType.subtract,
                            )
                        dst = o_d[n, kg * 128:(kg + 1) * 128, rb * RB: rb * RB + RB, :]
                        # stores go out via SWDGE (scalar engine) so they never
                        # queue ahead of the latency-critical input loads on
                        # the sync/HWDGE queues
                        nc.scalar.dma_start(dst, osb[:])

    nc.finalize()
    return nc


def _prep_weights(weight, mode):
    dt = mybir.dt
    xdt = dt.float8e4 if mode == "fp8" else dt.bfloat16
    sgn = np.sign(weight.astype(np.float32))
    w4 = sgn.reshape(K, 2, 128, 3, 3)          # [k, i, p, ty, tx]
    arr = w4.transpose(2, 3, 4, 1, 0)          # [p, ty, tx, i, k]
    arr = np.ascontiguousarray(arr).reshape(128, 9 * 2 * 256)
    return arr.astype(mybir.dt.np(xdt))


def kernel(x, weight, bias, _profile=False, _trace_kwargs=None):
    mode = "fp8" if USE_FP8 else "bf16"
    x = np.asarray(x, dtype=np.float32)
    weight = np.asarray(weight, dtype=np.float32)
    bias = np.asarray(bias, dtype=np.float32)
    with_bias = bool(np.any(bias != 0.0))

    key = (mode, with_bias)
    if key not in _cache:
        _cache[key] = _build(mode, with_bias)
    nc = _cache[key]

    wsgn = _prep_weights(weight, mode)
    in_maps = []
    for c in range(N_CORES):
        m = {
            "xs": np.ascontiguousarray(x[c * N_PER:(c + 1) * N_PER]),
            "wsgn": wsgn,
        }
        if with_bias:
            m["bhalf"] = np.ascontiguousarray(
                (bias.reshape(2, 128).T * 0.5).astype(np.float32)
            )
        in_maps.append(m)

    res = run_bass_kernel_spmd(
        nc, in_maps, core_ids=list(range(N_CORES)),
        trace=_profile, **(_trace_kwargs or {}),
    )
    out = np.concatenate([res.results[c]["out"] for c in range(N_CORES)], axis=0)
    if _profile:
        kernel.last_exec_ns = res.exec_time_ns
        kernel.last_results = res
    return out
